# revision 13
# baseline (speedup 1.0000x reference)
"""Causal self-attention (B=4, T=2048, C=1024, 16 heads x d=64) on 8 trn2 NeuronCores.

Strategy: tensor-parallel over heads — core i owns heads (2i, 2i+1).
Everything on-device runs in feature-major ("transposed") layout:
  xT [C, B*T] (host pre-transposes once) ->
  qT/kT [128=2*64 feats, T] per batch, vT -> V via PE transpose,
  S^T = K Q^T blocks [128 k, 512 q] (row-packed: both heads concurrently,
  diagonal blocks trimmed to the causally reachable column range),
  P^T = exp(S^T/8) with causal zeroing via affine_select,
  y^T [65, 512] = [V | ones]^T P^T  (ones column makes row 64 the softmax
  denominator, accumulated over k-blocks in PSUM).

Schedule (the point of this version): the k-loop is software-pipelined so
the PE never sits behind the Act engine's exp stream —
  per block: QK(kb) -> [filler unit] -> AV(kb-1),
i.e. AV lags QK by one block, and the gap the exp would otherwise leave in
the PE stream is filled with the PREVIOUS batch's finish work (broadcast +
projection matmuls, which only touch the ps1 PSUM slots and the DVE/Act
queues that have slack).  This keeps the PE dense through the attention
phase, which both removes the stall and keeps the HAM clock-gate at 8/8
(the baseline spent ~40% of its span at 4/8 because the alternating
QK->exp->AV chain let the PE idle).  Batch 0 has no previous finish work,
so scratch matmuls stand in as filler.
Per-qc streaming finish: after each qc's k-loop the y PSUM is drained to
SBUF in ONE DVE copy and the denominator row shipped to DRAM; per-batch
half-gathers (after qc1 and qc3) bounce the rows back partition-split so a
single fast approximate reciprocal covers them.  Each finish (anchored one
batch later, popped as k-loop filler) broadcasts 1/den across 64 partitions
via a one-hot-row matmul, normalizes into yT (DVE), and streams that
q-range's projection chunk through the ps1 accumulators with DVE casts.
qkv runs as a dense PE phase per batch; its bias-adds and V-transpose
copies sit on the (otherwise idle) Act engine so the DVE keeps its slack
for the k-loop casts.  The PE is kept dense from t=0 (HAM warmup matmuls,
dummy-padded cold-start qkv).
Host sums the 8 partial projections and adds b_proj.
"""

import sys

if "/opt/trn_rl_repo" not in sys.path:
    sys.path.insert(0, "/opt/trn_rl_repo")

import contextlib
import ctypes
import types

import numpy as np

import concourse.bass as bass
import concourse.mybir as mybir
import concourse.tile as tile
from concourse.bass_utils import run_bass_kernel_spmd

B, T, C = 4, 2048, 1024
N_HEAD, D = 16, 64
NCORES = 8
F32 = mybir.dt.float32

# matmul operand dtype: "fp32" (bit-exact, 4 cyc/row) or "fp32r" (1 cyc/row at
# free-dim >= 256, reduced-precision PE read)
DT_MM = "fp32r"
TRACE = False  # test.py flips this for profiled runs

_SO_PATH = "/opt/axon/libaxon_pjrt.so"


# ---------------------------------------------------------------------------
# Environment shims: (1) register the NTFF profile hook trn_boot could not
# install (image's antenv lacks axon_hooks); (2) this walrus build caps sem
# waits per instruction, but Tile's tail drain carries one wait per active
# proc — spread them over single-wait SP NOPs instead.
# ---------------------------------------------------------------------------
def _install_ntff_hook():
    if "antenv.axon_hooks" in sys.modules:
        return
    state = {"hook": None}

    def set_hook(h):
        state["hook"] = h

    def get_hook():
        return state["hook"]

    mod = types.ModuleType("antenv.axon_hooks")
    mod.set_axon_ntff_profile_hook = set_hook
    mod.get_axon_ntff_profile_hook = get_hook
    sys.modules["antenv.axon_hooks"] = mod
    import antenv

    antenv.axon_hooks = mod

    try:
        lib = ctypes.CDLL(_SO_PATH)
    except OSError:
        return
    if not hasattr(lib, "axon_start_nrt_profile"):
        return
    lib.axon_start_nrt_profile.argtypes = [
        ctypes.POINTER(ctypes.c_int64),
        ctypes.c_size_t,
    ]
    lib.axon_start_nrt_profile.restype = ctypes.c_int64
    lib.axon_stop_nrt_profile.argtypes = [ctypes.c_char_p]
    lib.axon_stop_nrt_profile.restype = ctypes.c_int64

    @contextlib.contextmanager
    def _hook_cm(output_dir, device_ids):
        import jax

        jax.devices()
        if device_ids:
            ids = (ctypes.c_int64 * len(device_ids))(*device_ids)
            rc = lib.axon_start_nrt_profile(ids, len(device_ids))
        else:
            rc = lib.axon_start_nrt_profile(None, 0)
        if rc != 0:
            raise RuntimeError(f"axon_start_nrt_profile rc={rc}")
        try:
            yield
        finally:
            n = lib.axon_stop_nrt_profile(str(output_dir).encode())
            if n < 0:
                raise RuntimeError(f"axon_stop_nrt_profile rc={n}")
            print(f"profile: {n} file(s) written to {output_dir}", file=sys.stderr)

    set_hook(_hook_cm)


def _patch_tile_tail_drain():
    from concourse.vector_clock import ScopedClock, VectorClock

    if getattr(tile.TileContext, "_drain_patch", False):
        return

    def patched(self, tick_clock, wait_clock):
        vc = tick_clock.global_clock
        n = len(vc)
        for proc in range(n):
            t = vc[proc]
            if t <= 0:
                continue
            sub = VectorClock([t if i == proc else 0 for i in range(n)])
            nop = self.nc.sync.nop(nofuse=True)
            wait_clock.add_sem_waits(nop.ins, ScopedClock({None: sub}))
        # Same tail as the original _drain_and_barrier, minus the multi-wait
        # drain — the NOP chain above already waited on every proc.
        self.nc.sync.drain()
        self.nc.all_engine_barrier()
        assert self.sems is not None
        popped = self.nc._tile_sem_poison_stack.pop()
        assert popped is self._sem_poison
        self.nc.clear_and_free_semaphores(list(self.sems.allocated().values()))
        self.nc.all_engine_barrier()

    tile.TileContext._drain_and_barrier = patched
    tile.TileContext._drain_patch = True


_install_ntff_hook()
_patch_tile_tail_drain()


def _split_waits(nc, limit=1):
    """This walrus build rejects instructions carrying more than ~2 sem waits.
    Spill excess waits onto preceding same-engine NOPs (program order on the
    issuing engine preserves the blocking semantics exactly)."""
    k = 0
    for fn in nc.m.functions:
        for bb in fn.blocks:
            new = []
            for ins in bb.instructions:
                si = ins.sync_info
                waits = list(si.on_wait) if si and si.on_wait else []
                if len(waits) > limit:
                    for w in waits[:-limit]:
                        nop = mybir.InstNoOp(name=f"I-wsplit-{k}")
                        k += 1
                        nop.engine = ins.engine
                        nop.sync_info = mybir.SyncInfo(on_wait=[w], on_update=[])
                        new.append(nop)
                    ins.sync_info = mybir.SyncInfo(
                        on_wait=waits[-limit:],
                        on_update=list(si.on_update) if si.on_update else [],
                    )
                new.append(ins)
            bb.instructions = new


def _op_dtype():
    return {
        "fp32": mybir.dt.float32,
        "fp32r": mybir.dt.float32r,
        "bf16": mybir.dt.bfloat16,
    }[DT_MM]


def _op_npdtype():
    return mybir.dt.np(_op_dtype())


def build_nc():
    DT = _op_dtype()
    BF16 = mybir.dt.bfloat16
    nc = bass.Bass()
    xT = nc.declare_dram_parameter("xT", [C, B * T], DT, isOutput=False)
    wqkv = nc.declare_dram_parameter("wqkv", [C, 384], DT, isOutput=False)
    bqkv = nc.declare_dram_parameter("bqkv", [128, 3], F32, isOutput=False)
    wproj = nc.declare_dram_parameter("wproj", [128, C], DT, isOutput=False)
    ident = nc.declare_dram_parameter("ident", [128, 128], DT, isOutput=False)
    ehot = nc.declare_dram_parameter("ehot", [8, 8, 64], DT, isOutput=False)
    outT = nc.declare_dram_parameter("outT", [C, B * T], BF16, isOutput=True)

    EXP = mybir.ActivationFunctionType.Exp

    wide = mybir.dt.size(DT) > 2  # debug dtypes need smaller pools to fit SBUF
    with tile.TileContext(nc) as tc:
        with contextlib.ExitStack() as ctx:
            singles = ctx.enter_context(tc.tile_pool(name="singles", bufs=1))
            xpool = ctx.enter_context(tc.tile_pool(name="xpool", bufs=9))
            qkv_sb = ctx.enter_context(tc.tile_pool(name="qkv_sb", bufs=1))
            vt_pool = ctx.enter_context(tc.tile_pool(name="vtp", bufs=1))
            vaug_p = ctx.enter_context(tc.tile_pool(name="vaug", bufs=1))
            pt_pool = ctx.enter_context(tc.tile_pool(name="ptp", bufs=4))
            yt_pool = ctx.enter_context(tc.tile_pool(name="ytp", bufs=1 if wide else 2))
            yub_p = ctx.enter_context(tc.tile_pool(name="yub", bufs=8))
            sm_pool = ctx.enter_context(tc.tile_pool(name="smp", bufs=2))
            dscr = ctx.enter_context(tc.tile_pool(name="dscr", bufs=2, space="DRAM"))
            ost_pool = ctx.enter_context(tc.tile_pool(name="ost", bufs=2))
            # PSUM (8 banks): s [128,2,512] x2 bufs = 4 banks, y01 [65,2,512]
            # = 2 banks, ps1 [128,512] x2 = 2 banks (qkv accum / transposes /
            # warmup / filler broadcast+proj accumulators)
            ps1 = ctx.enter_context(tc.tile_pool(name="ps1", bufs=2, space="PSUM"))
            ps_s = ctx.enter_context(tc.tile_pool(name="ps_s", bufs=2, space="PSUM"))
            ps_y = ctx.enter_context(tc.tile_pool(name="ps_y", bufs=1, space="PSUM"))

            # tiny ident first (nothing ahead of it on the queue), then wq
            # (gates all qkv matmuls); x(0) slabs follow in emit_qkv(0);
            # bq/wp land before their first use.
            id_sb = singles.tile([128, 128], DT)
            nc.sync.dma_start(out=id_sb, in_=ident[:, :])
            wq_sb = singles.tile([128, 8, 384], DT)
            nc.sync.dma_start(out=wq_sb, in_=wqkv.rearrange("(a p) f -> p a f", p=128))
            bq_sb = singles.tile([128, 3], F32)
            nc.sync.dma_start(out=bq_sb, in_=bqkv[:, :])
            wp_sb = singles.tile([128, C], DT)
            nc.sync.dma_start(out=wp_sb, in_=wproj[:, :])

            # scratch first: the HAM warmup matmuls depend only on this chain
            scratchF = singles.tile([128, 512], F32)
            nc.vector.memset(scratchF, 0.0)
            scratch = singles.tile([128, 512], DT)
            nc.vector.tensor_copy(scratch, scratchF)
            ones_col = singles.tile([128, 16, 1], F32)
            nc.vector.memset(ones_col, 1.0)
            # one-hot-row selectors (host-supplied): matmul(lhsT=erows[:, r, :],
            # rhs=[n, 512]) broadcasts row r of the rhs across 64 partitions
            erows = singles.tile([8, 8, 64], DT)
            nc.sync.dma_start(out=erows, in_=ehot[:, :, :])

            # HAM warmup: ~20 dependency-free matmuls keep the PE busy from
            # t=0 so the clock gate reaches 8/8 before the real work starts
            # (x DMAs are in flight meanwhile).
            warm = ps1.tile([128, 512], F32, tag="ps1")
            # first few warmups read the (tiny, first-issued) ident DMA so the
            # PE starts before the scratch memset/copy chain clears the DVE
            for _ in range(4):
                nc.tensor.matmul(
                    warm[:, 0:128], lhsT=id_sb, rhs=id_sb, start=True, stop=True
                )
            for _ in range(18):
                nc.tensor.matmul(
                    warm, lhsT=scratch[:, 0:128], rhs=scratch, start=True, stop=True
                )

            state = {}

            def emit_qkv(b):
                qT = qkv_sb.tile([128, T], DT, tag="qT")
                kT = qkv_sb.tile([128, T], DT, tag="kT")
                vT = vt_pool.tile([128, T], DT, tag="vT")
                if b == 0:
                    # cold start: split each slab into two half DMAs so the
                    # first matmul groups start after ~4 MB, not 8 MB
                    slabs = []
                    for c in range(8):
                        sl = xpool.tile([128, 2048], DT, tag="xslab")
                        nc.sync.dma_start(
                            out=sl[:, 0:1024],
                            in_=xT[c * 128 : (c + 1) * 128, 0:1024],
                        )
                        slabs.append(sl)
                    for c in range(8):
                        nc.sync.dma_start(
                            out=slabs[c][:, 1024:2048],
                            in_=xT[c * 128 : (c + 1) * 128, 1024:2048],
                        )
                else:
                    slabs = state[b].pop("slabs")
                for tch in range(4):
                    for m, dst in enumerate((qT, kT, vT)):
                        ps = ps1.tile([128, 512], F32, tag="ps1")
                        if b == 0:
                            # cold start is DMA-paced: pad the PE stream with
                            # free dummy matmuls (into this group's own bank,
                            # reset by the c==0 start below) so the HAM clock
                            # gate never sees an idle window and stays at 8/8
                            for _ in range(8 if (tch, m) == (0, 0) else 3):
                                nc.tensor.matmul(
                                    ps,
                                    lhsT=scratch[:, 0:128],
                                    rhs=scratch,
                                    start=True,
                                    stop=True,
                                )
                        for c in range(8):
                            nc.tensor.matmul(
                                ps,
                                lhsT=wq_sb[:, c, m * 128 : (m + 1) * 128],
                                rhs=slabs[c][:, tch * 512 : (tch + 1) * 512],
                                start=(c == 0),
                                stop=(c == 7),
                            )
                        nc.vector.tensor_scalar_add(
                            dst[:, tch * 512 : (tch + 1) * 512], ps, bq_sb[:, m : m + 1]
                        )
                # vT -> V (token-major) + ones column
                va0 = vaug_p.tile([128, 16, 65], DT, tag="va0")
                va1 = vaug_p.tile([128, 16, 65], DT, tag="va1")
                nc.vector.tensor_copy(va0[:, :, 64:65], ones_col)
                nc.vector.tensor_copy(va1[:, :, 64:65], ones_col)
                for tt in range(16):
                    tp = ps1.tile([128, 128], DT, tag="ps1")
                    nc.tensor.transpose(tp, vT[:, tt * 128 : (tt + 1) * 128], id_sb)
                    nc.vector.tensor_copy(va0[:, tt, 0:64], tp[:, 0:64])
                    nc.vector.tensor_copy(va1[:, tt, 0:64], tp[:, 64:128])
                yT = yt_pool.tile([128, T], DT, tag="yT")
                state[b] = {"qT": qT, "kT": kT, "va0": va0, "va1": va1, "yT": yT}

            def emit_kloop(b, qc, filler=None):
                # software-pipelined: AV lags QK by one block so the PE never
                # waits on that block's exp; one filler unit (previous batch's
                # finish work) per block keeps the PE dense while Act paces.
                st = state[b]
                qT, kT, va0, va1 = st["qT"], st["kT"], st["va0"], st["va1"]
                y01 = ps_y.tile([65, 2, 512], F32, tag="y01")
                nkb = 4 * qc + 4
                pts = {}

                def emit_av(kb):
                    pt, lo = pts.pop(kb)
                    nc.tensor.matmul(
                        y01[:, 0, lo:512],
                        lhsT=va0[:, kb, :],
                        rhs=pt[:, 0, lo:512],
                        start=(kb == 0),
                        stop=(kb == nkb - 1),
                    )
                    nc.tensor.matmul(
                        y01[:, 1, lo:512],
                        lhsT=va1[:, kb, :],
                        rhs=pt[:, 1, lo:512],
                        start=(kb == 0),
                        stop=(kb == nkb - 1),
                    )

                for kb in range(nkb):
                    s = ps_s.tile([128, 2, 512], F32, tag="s")
                    # for diagonal blocks only columns q >= (kb-4qc)*128 are
                    # causally reachable; skip the rest entirely (S included)
                    j = max(kb - 4 * qc, 0) if kb >= 4 * qc else 0
                    lo = j * 128
                    nc.tensor.matmul(
                        s[:, 0, lo:512],
                        lhsT=kT[0:64, kb * 128 : (kb + 1) * 128],
                        rhs=qT[0:64, qc * 512 + lo : (qc + 1) * 512],
                        start=True,
                        stop=True,
                    )
                    nc.tensor.matmul(
                        s[:, 1, lo:512],
                        lhsT=kT[64:128, kb * 128 : (kb + 1) * 128],
                        rhs=qT[64:128, qc * 512 + lo : (qc + 1) * 512],
                        start=True,
                        stop=True,
                    )
                    pt = pt_pool.tile([128, 2, 512], DT, tag="pt")
                    nc.scalar.activation(pt[:, :, lo:512], s[:, :, lo:512], EXP, scale=0.125)
                    if kb >= 4 * qc:
                        nc.gpsimd.affine_select(
                            out=pt[:, :, lo : lo + 128],
                            in_=pt[:, :, lo : lo + 128],
                            pattern=[[0, 2], [1, 128]],
                            base=0,
                            channel_multiplier=-1,
                            compare_op=mybir.AluOpType.is_ge,
                            fill=0.0,
                        )
                    pts[kb] = (pt, lo)
                    if filler is not None and kb >= 2:
                        next(filler, None)
                    if kb >= 1:
                        emit_av(kb - 1)
                if filler is not None:
                    next(filler, None)
                emit_av(nkb - 1)
                st[("y", qc)] = y01

            def emit_drain(b, qc):
                # release y PSUM promptly (single DVE copy); ship the
                # denominator row (partition 64) to DRAM — per-batch half
                # bounces re-partition the rows so one reciprocal covers them.
                st = state[b]
                y01 = st.pop(("y", qc))
                yub = yub_p.tile([65, 2, 512], F32, tag="yub")
                nc.vector.tensor_copy(yub, y01)
                nc.sync.dma_start(out=st["dden"][qc], in_=yub[64:65, :, :])
                st[("yub", qc)] = yub

            def emit_gather(b, rows):
                # bounce back from DRAM partition-split, fast-approx
                # reciprocal (~18 correct bits, plenty for the 2e-2 gate)
                st = state[b]
                lo, hi = rows
                n = hi - lo
                dsum = sm_pool.tile([n, 512], F32, tag="dsum")
                nc.sync.dma_start(
                    out=dsum, in_=st["dden"].rearrange("a h q -> (a h) q")[lo:hi]
                )
                rinv = sm_pool.tile([n, 512], F32, tag="rinv")
                nc.vector.reciprocal(rinv, dsum)
                rdt = sm_pool.tile([n, 512], DT, tag="rdt", bufs=4)
                nc.vector.tensor_copy(rdt, rinv)
                st[("rdt", lo)] = rdt

            def _rdt_rows(b, qc):
                st = state[b]
                if b == B - 1 and qc >= 2:
                    return st[("rdt", 4)], 2 * (qc - 2)
                return st[("rdt", 0)], 2 * qc

            def part2_chunks(b, qc):
                # generator form for interleaving into a k-loop: uses only ps1
                # slots (free during attention) so it never contends with the
                # k-loop's s/y PSUM rotation; one proj MM + cast per chunk
                st = state[b]
                yub = st.pop(("yub", qc))
                yT = st["yT"]
                rdt, r0 = _rdt_rows(b, qc)
                n = rdt.shape[0]
                for h in range(2):
                    rb = ps1.tile([64, 512], F32, tag="ps1")
                    nc.tensor.matmul(
                        rb,
                        lhsT=erows[0:n, r0 + h, :],
                        rhs=rdt,
                        start=True,
                        stop=True,
                    )
                    nc.vector.tensor_mul(
                        yT[h * 64 : (h + 1) * 64, qc * 512 : (qc + 1) * 512],
                        yub[0:64, h, :],
                        rb,
                    )
                yield
                osb = None
                for mt in range(8):
                    if mt % 4 == 0:
                        osb = ost_pool.tile(
                            [128, 4, 512], mybir.dt.bfloat16, tag="osb"
                        )
                    o = ps1.tile([128, 512], F32, tag="ps1")
                    nc.tensor.matmul(
                        o,
                        lhsT=wp_sb[:, mt * 128 : (mt + 1) * 128],
                        rhs=yT[:, qc * 512 : (qc + 1) * 512],
                        start=True,
                        stop=True,
                    )
                    # DVE only: an Act cast here would head-of-line block the
                    # enclosing k-loop's exp stream (in-order engine)
                    nc.vector.tensor_copy(osb[:, mt % 4, :], o)
                    if mt % 4 == 3:
                        g = mt // 4
                        nc.sync.dma_start(
                            out=outT[
                                g * 512 : (g + 1) * 512,
                                b * T + qc * 512 : b * T + (qc + 1) * 512,
                            ].rearrange("(a p) q -> p a q", p=128),
                            in_=osb,
                        )
                    yield

            def batch_filler(b):
                # previous batch's four finishes, consumed one unit per
                # k-loop block of the NEXT batch (36 units vs 36 slots)
                for qc in range(4):
                    yield from part2_chunks(b, qc)

            def dummy_filler():
                # batch 0 has no previous finish work; scratch matmuls keep
                # the PE stream dense enough to hold the HAM clock at 8/8
                while True:
                    ps = ps1.tile([128, 512], F32, tag="ps1")
                    nc.tensor.matmul(
                        ps, lhsT=scratch[:, 0:128], rhs=scratch, start=True, stop=True
                    )
                    yield

            def emit_part2(b, qc, tail=False, only=None):
                # tail-only finish: broadcast 1/den across 64 partitions via a
                # K=n matmul, normalize into yT and run the projection chunk.
                # Proj accumulators rotate s/ps1/y01 PSUM slots so the
                # PSUM->bf16 casts (split DVE/Act) never gate the PE.
                st = state[b]
                yT = st["yT"]
                if only != "b":
                    yub = st.pop(("yub", qc))
                    rdt, r0 = _rdt_rows(b, qc)
                    n = rdt.shape[0]
                    rb = ps_s.tile([64, 2, 512], F32, tag="s")
                    for h in range(2):
                        nc.tensor.matmul(
                            rb[:, h, :],
                            lhsT=erows[0:n, r0 + h, :],
                            rhs=rdt,
                            start=True,
                            stop=True,
                        )
                        nc.vector.tensor_mul(
                            yT[h * 64 : (h + 1) * 64, qc * 512 : (qc + 1) * 512],
                            yub[0:64, h, :],
                            rb[:, h, :],
                        )
                if only == "a":
                    return
                slot = [
                    (ps_s, "s"),
                    (ps1, "ps1"),
                    (ps1, "ps1"),
                    (ps_s, "s"),
                    (ps_y, "y01"),
                    (ps1, "ps1"),
                    (ps1, "ps1"),
                    (ps_s, "s"),
                ]
                # the very last finish ships half-groups so the final DMA (and
                # the end-of-kernel drain behind it) starts as early as possible
                per_dma = 2 if (tail and qc == 3) else 4
                for g in range(2):
                    osb = ost_pool.tile([128, 4, 512], mybir.dt.bfloat16, tag="osb")
                    for i in range(4):
                        mt = 4 * g + i
                        pool, tag = slot[mt]
                        o = pool.tile([128, 512], F32, tag=tag)
                        nc.tensor.matmul(
                            o,
                            lhsT=wp_sb[:, mt * 128 : (mt + 1) * 128],
                            rhs=yT[:, qc * 512 : (qc + 1) * 512],
                            start=True,
                            stop=True,
                        )
                        # tail: no exps follow — weight the cast split toward
                        # Act (3/5) since DVE also carries the muls
                        if tail and mt in (1, 3, 5, 6, 7):
                            nc.scalar.copy(osb[:, i, :], o)
                        else:
                            nc.vector.tensor_copy(osb[:, i, :], o)
                        if (i + 1) % per_dma == 0:
                            j = i + 1 - per_dma
                            nc.sync.dma_start(
                                out=outT[
                                    g * 512 + j * 128 : g * 512 + (i + 1) * 128,
                                    b * T + qc * 512 : b * T + (qc + 1) * 512,
                                ].rearrange("(a p) q -> p a q", p=128),
                                in_=osb[:, j : i + 1, :],
                            )

            for b in range(B):
                emit_qkv(b)
                dden = dscr.tile([4, 2, 512], F32, tag="dden")
                state[b]["dden"] = dden
                if b + 1 < B:
                    # pre-emit next batch's x DMAs so they sit ahead of the
                    # proj-out DMAs in the Sync stream (in-order issue engine)
                    nxt = []
                    for c in range(8):
                        sl = xpool.tile([128, 2048], DT, tag="xslab")
                        nc.sync.dma_start(
                            out=sl,
                            in_=xT[c * 128 : (c + 1) * 128, (b + 1) * T : (b + 2) * T],
                        )
                        nxt.append(sl)
                    state[b + 1] = {"slabs": nxt}
                filler = batch_filler(b - 1) if b > 0 else dummy_filler()
                for qc in range(4):
                    emit_kloop(b, qc, filler)
                    emit_drain(b, qc)
                    if b == B - 1 and qc == 1:
                        emit_gather(b, (0, 4))
                if b < B - 1:
                    emit_gather(b, (0, 8))
                if b > 0:
                    # flush any leftover finish units (normally none)
                    for _ in filler:
                        pass
            # tail: interleave the last batch's four finishes (a = bcast+mul,
            # b = proj) so the PE never idles long enough to drop the HAM clock
            emit_part2(B - 1, 0, tail=True, only="a")
            emit_part2(B - 1, 1, tail=True, only="a")
            emit_part2(B - 1, 0, tail=True, only="b")
            emit_gather(B - 1, (4, 8))
            emit_part2(B - 1, 1, tail=True, only="b")
            emit_part2(B - 1, 2, tail=True, only="a")
            emit_part2(B - 1, 2, tail=True, only="b")
            emit_part2(B - 1, 3, tail=True, only="a")
            emit_part2(B - 1, 3, tail=True, only="b")

    _split_waits(nc)
    return nc


_nc_cache = None


def kernel(x, W_qkv, b_qkv, W_proj, b_proj):
    global _nc_cache
    x = np.ascontiguousarray(np.asarray(x, dtype=np.float32))
    W_qkv = np.asarray(W_qkv, dtype=np.float32)
    b_qkv = np.asarray(b_qkv, dtype=np.float32)
    W_proj = np.asarray(W_proj, dtype=np.float32)
    b_proj = np.asarray(b_proj, dtype=np.float32)

    npdt = _op_npdtype()
    xT = np.ascontiguousarray(x.reshape(B * T, C).T).astype(npdt)
    ident = np.eye(128, dtype=np.float32).astype(npdt)
    ehot = np.ascontiguousarray(
        np.broadcast_to(np.eye(8, dtype=np.float32)[:, :, None], (8, 8, 64))
    ).astype(npdt)

    in_maps = []
    for i in range(NCORES):
        s = slice(128 * i, 128 * (i + 1))
        wq = np.ascontiguousarray(
            np.concatenate(
                [W_qkv[:, s], W_qkv[:, 1024:2048][:, s], W_qkv[:, 2048:3072][:, s]],
                axis=1,
            )
        ).astype(npdt)
        bq = np.ascontiguousarray(
            np.stack([b_qkv[0:1024][s], b_qkv[1024:2048][s], b_qkv[2048:3072][s]], axis=1)
        )
        wp = np.ascontiguousarray(W_proj[s, :]).astype(npdt)
        in_maps.append(
            {"xT": xT, "wqkv": wq, "bqkv": bq, "wproj": wp, "ident": ident, "ehot": ehot}
        )

    if _nc_cache is None:
        _nc_cache = build_nc()
    res = run_bass_kernel_spmd(_nc_cache, in_maps, list(range(NCORES)), trace=TRACE)
    kernel.last_result = res

    acc = np.zeros((C, B * T), dtype=np.float32)
    for r in res.results:
        acc += np.asarray(r["outT"], dtype=np.float32)
    out = acc.T.reshape(B, T, C) + b_proj
    return out.astype(np.float32)


# revision 15
# speedup vs baseline: 1.2852x; 1.2852x over previous
"""Causal self-attention (B=4, T=2048, C=1024, 16 heads x d=64) on 8 trn2 NeuronCores.

Strategy: tensor-parallel over heads — core i owns heads (2i, 2i+1).
Everything on-device runs in feature-major ("transposed") layout:
  xT [C, B*T] (host pre-transposes once) ->
  qT/kT [128=2*64 feats, T] per batch, vT -> V via PE transpose,
  S^T = K Q^T blocks [128 k, 512 q] (row-packed: both heads concurrently,
  diagonal blocks trimmed to the causally reachable column range),
  P^T = exp(S^T/8) with causal zeroing via affine_select,
  y^T [65, 512] = [V | ones]^T P^T  (ones column makes row 64 the softmax
  denominator, accumulated over k-blocks in PSUM).

Schedule (the point of this version): the k-loop is software-pipelined so
the PE never sits behind the Act engine's exp stream —
  per block: QK(kb) -> [filler unit] -> AV(kb-1),
i.e. AV lags QK by one block, and the gap the exp would otherwise leave in
the PE stream is filled with the PREVIOUS batch's finish work (broadcast +
projection matmuls, popped one unit per block via the part2_chunks
generators; they only touch the ps1 PSUM slots and the DVE queue, which
have slack during attention).  This keeps the PE dense through the
attention phase, which both removes the per-block QK->exp->AV stall and
keeps the HAM clock-gate at 8/8 (the previous version spent ~40% of its
span at 4/8 because the alternating chain let the PE idle).
Per-qc streaming finish: after each qc's k-loop the y PSUM is drained to
SBUF in ONE DVE copy ([65,2,512], freeing both y banks at once) and the
denominator row shipped to DRAM; one per-batch bounce re-partitions all 8
rows so a single reciprocal covers them (the last batch gathers its first
half early so the tail finishes never wait).  NOTE: the DMA issue sequence
of this bounce is timing-sensitive on this walrus build — a variant that
split the gather into per-half-batch reads (extra dsum DMA per batch)
deterministically corrupted the first batch's finishes on hardware while
passing CoreSim + race detector; keep the gather DMA pattern exactly as
the proven baseline emits it.
Each finish (anchored one batch later) broadcasts 1/den across 64
partitions via a one-hot-row matmul, normalizes into yT (DVE), and runs
that q-range's projection chunk through the ps1 accumulators with DVE
casts (an Act cast would head-of-line-block the exp stream).
The PE is kept dense from t=0 (HAM clock-gate warmup matmuls, dummy-padded
cold-start qkv) so the 8/8 clock ratio holds.
Host sums the 8 partial projections and adds b_proj.
"""

import sys

if "/opt/trn_rl_repo" not in sys.path:
    sys.path.insert(0, "/opt/trn_rl_repo")

import contextlib
import ctypes
import types

import numpy as np

import concourse.bass as bass
import concourse.mybir as mybir
import concourse.tile as tile
from concourse.bass_utils import run_bass_kernel_spmd

B, T, C = 4, 2048, 1024
N_HEAD, D = 16, 64
NCORES = 8
F32 = mybir.dt.float32

# matmul operand dtype: "fp32" (bit-exact, 4 cyc/row) or "fp32r" (1 cyc/row at
# free-dim >= 256, reduced-precision PE read)
DT_MM = "fp32r"
TRACE = False  # test.py flips this for profiled runs

_SO_PATH = "/opt/axon/libaxon_pjrt.so"


# ---------------------------------------------------------------------------
# Environment shims: (1) register the NTFF profile hook trn_boot could not
# install (image's antenv lacks axon_hooks); (2) this walrus build caps sem
# waits per instruction, but Tile's tail drain carries one wait per active
# proc — spread them over single-wait SP NOPs instead.
# ---------------------------------------------------------------------------
def _install_ntff_hook():
    if "antenv.axon_hooks" in sys.modules:
        return
    state = {"hook": None}

    def set_hook(h):
        state["hook"] = h

    def get_hook():
        return state["hook"]

    mod = types.ModuleType("antenv.axon_hooks")
    mod.set_axon_ntff_profile_hook = set_hook
    mod.get_axon_ntff_profile_hook = get_hook
    sys.modules["antenv.axon_hooks"] = mod
    import antenv

    antenv.axon_hooks = mod

    try:
        lib = ctypes.CDLL(_SO_PATH)
    except OSError:
        return
    if not hasattr(lib, "axon_start_nrt_profile"):
        return
    lib.axon_start_nrt_profile.argtypes = [
        ctypes.POINTER(ctypes.c_int64),
        ctypes.c_size_t,
    ]
    lib.axon_start_nrt_profile.restype = ctypes.c_int64
    lib.axon_stop_nrt_profile.argtypes = [ctypes.c_char_p]
    lib.axon_stop_nrt_profile.restype = ctypes.c_int64

    @contextlib.contextmanager
    def _hook_cm(output_dir, device_ids):
        import jax

        jax.devices()
        if device_ids:
            ids = (ctypes.c_int64 * len(device_ids))(*device_ids)
            rc = lib.axon_start_nrt_profile(ids, len(device_ids))
        else:
            rc = lib.axon_start_nrt_profile(None, 0)
        if rc != 0:
            raise RuntimeError(f"axon_start_nrt_profile rc={rc}")
        try:
            yield
        finally:
            n = lib.axon_stop_nrt_profile(str(output_dir).encode())
            if n < 0:
                raise RuntimeError(f"axon_stop_nrt_profile rc={n}")
            print(f"profile: {n} file(s) written to {output_dir}", file=sys.stderr)

    set_hook(_hook_cm)


def _patch_tile_tail_drain():
    from concourse.vector_clock import ScopedClock, VectorClock

    if getattr(tile.TileContext, "_drain_patch", False):
        return

    def patched(self, tick_clock, wait_clock):
        vc = tick_clock.global_clock
        n = len(vc)
        for proc in range(n):
            t = vc[proc]
            if t <= 0:
                continue
            sub = VectorClock([t if i == proc else 0 for i in range(n)])
            nop = self.nc.sync.nop(nofuse=True)
            wait_clock.add_sem_waits(nop.ins, ScopedClock({None: sub}))
        # Same tail as the original _drain_and_barrier, minus the multi-wait
        # drain — the NOP chain above already waited on every proc.
        self.nc.sync.drain()
        self.nc.all_engine_barrier()
        assert self.sems is not None
        popped = self.nc._tile_sem_poison_stack.pop()
        assert popped is self._sem_poison
        self.nc.clear_and_free_semaphores(list(self.sems.allocated().values()))
        self.nc.all_engine_barrier()

    tile.TileContext._drain_and_barrier = patched
    tile.TileContext._drain_patch = True


_install_ntff_hook()
_patch_tile_tail_drain()


def _split_waits(nc, limit=1):
    """This walrus build rejects instructions carrying more than ~2 sem waits.
    Spill excess waits onto preceding same-engine NOPs (program order on the
    issuing engine preserves the blocking semantics exactly)."""
    k = 0
    for fn in nc.m.functions:
        for bb in fn.blocks:
            new = []
            for ins in bb.instructions:
                si = ins.sync_info
                waits = list(si.on_wait) if si and si.on_wait else []
                if len(waits) > limit:
                    for w in waits[:-limit]:
                        nop = mybir.InstNoOp(name=f"I-wsplit-{k}")
                        k += 1
                        nop.engine = ins.engine
                        nop.sync_info = mybir.SyncInfo(on_wait=[w], on_update=[])
                        new.append(nop)
                    ins.sync_info = mybir.SyncInfo(
                        on_wait=waits[-limit:],
                        on_update=list(si.on_update) if si.on_update else [],
                    )
                new.append(ins)
            bb.instructions = new


def _op_dtype():
    return {
        "fp32": mybir.dt.float32,
        "fp32r": mybir.dt.float32r,
        "bf16": mybir.dt.bfloat16,
    }[DT_MM]


def _op_npdtype():
    return mybir.dt.np(_op_dtype())


def build_nc():
    DT = _op_dtype()
    BF16 = mybir.dt.bfloat16
    nc = bass.Bass()
    xT = nc.declare_dram_parameter("xT", [C, B * T], DT, isOutput=False)
    wqkv = nc.declare_dram_parameter("wqkv", [C, 384], DT, isOutput=False)
    bqkv = nc.declare_dram_parameter("bqkv", [128, 3], F32, isOutput=False)
    wproj = nc.declare_dram_parameter("wproj", [128, C], DT, isOutput=False)
    ident = nc.declare_dram_parameter("ident", [128, 128], DT, isOutput=False)
    ehot = nc.declare_dram_parameter("ehot", [8, 8, 64], DT, isOutput=False)
    outT = nc.declare_dram_parameter("outT", [C, B * T], BF16, isOutput=True)

    EXP = mybir.ActivationFunctionType.Exp

    wide = mybir.dt.size(DT) > 2  # debug dtypes need smaller pools to fit SBUF
    with tile.TileContext(nc) as tc:
        with contextlib.ExitStack() as ctx:
            singles = ctx.enter_context(tc.tile_pool(name="singles", bufs=1))
            xpool = ctx.enter_context(tc.tile_pool(name="xpool", bufs=9))
            qkv_sb = ctx.enter_context(tc.tile_pool(name="qkv_sb", bufs=1))
            vt_pool = ctx.enter_context(tc.tile_pool(name="vtp", bufs=1))
            vaug_p = ctx.enter_context(tc.tile_pool(name="vaug", bufs=1))
            pt_pool = ctx.enter_context(tc.tile_pool(name="ptp", bufs=4))
            yt_pool = ctx.enter_context(tc.tile_pool(name="ytp", bufs=1 if wide else 2))
            yub_p = ctx.enter_context(tc.tile_pool(name="yub", bufs=8))
            sm_pool = ctx.enter_context(tc.tile_pool(name="smp", bufs=2))
            dscr = ctx.enter_context(tc.tile_pool(name="dscr", bufs=2, space="DRAM"))
            ost_pool = ctx.enter_context(tc.tile_pool(name="ost", bufs=2))
            # PSUM (8 banks): s [128,2,512] x2 bufs = 4 banks, y01 [65,2,512]
            # = 2 banks, ps1 [128,512] x2 = 2 banks (qkv accum / transposes /
            # warmup / filler broadcast+proj accumulators)
            ps1 = ctx.enter_context(tc.tile_pool(name="ps1", bufs=2, space="PSUM"))
            ps_s = ctx.enter_context(tc.tile_pool(name="ps_s", bufs=2, space="PSUM"))
            ps_y = ctx.enter_context(tc.tile_pool(name="ps_y", bufs=1, space="PSUM"))

            # tiny ident first (nothing ahead of it on the queue), then wq
            # (gates all qkv matmuls); x(0) slabs follow in emit_qkv(0);
            # bq/wp land before their first use.
            id_sb = singles.tile([128, 128], DT)
            nc.sync.dma_start(out=id_sb, in_=ident[:, :])
            wq_sb = singles.tile([128, 8, 384], DT)
            nc.sync.dma_start(out=wq_sb, in_=wqkv.rearrange("(a p) f -> p a f", p=128))
            bq_sb = singles.tile([128, 3], F32)
            nc.sync.dma_start(out=bq_sb, in_=bqkv[:, :])
            wp_sb = singles.tile([128, C], DT)
            nc.sync.dma_start(out=wp_sb, in_=wproj[:, :])

            # scratch first: the HAM warmup matmuls depend only on this chain
            scratchF = singles.tile([128, 512], F32)
            nc.vector.memset(scratchF, 0.0)
            scratch = singles.tile([128, 512], DT)
            nc.vector.tensor_copy(scratch, scratchF)
            ones_col = singles.tile([128, 16, 1], F32)
            nc.vector.memset(ones_col, 1.0)
            # one-hot-row selectors (host-supplied): matmul(lhsT=erows[:, r, :],
            # rhs=[n, 512]) broadcasts row r of the rhs across 64 partitions
            erows = singles.tile([8, 8, 64], DT)
            nc.sync.dma_start(out=erows, in_=ehot[:, :, :])

            # HAM warmup: ~20 dependency-free matmuls keep the PE busy from
            # t=0 so the clock gate reaches 8/8 before the real work starts
            # (x DMAs are in flight meanwhile).
            warm = ps1.tile([128, 512], F32, tag="ps1")
            # first few warmups read the (tiny, first-issued) ident DMA so the
            # PE starts before the scratch memset/copy chain clears the DVE
            for _ in range(4):
                nc.tensor.matmul(
                    warm[:, 0:128], lhsT=id_sb, rhs=id_sb, start=True, stop=True
                )
            for _ in range(18):
                nc.tensor.matmul(
                    warm, lhsT=scratch[:, 0:128], rhs=scratch, start=True, stop=True
                )

            state = {}

            def emit_qkv(b):
                qT = qkv_sb.tile([128, T], DT, tag="qT")
                kT = qkv_sb.tile([128, T], DT, tag="kT")
                vT = vt_pool.tile([128, T], DT, tag="vT")
                if b == 0:
                    # cold start: split each slab into two half DMAs so the
                    # first matmul groups start after ~4 MB, not 8 MB
                    slabs = []
                    for c in range(8):
                        sl = xpool.tile([128, 2048], DT, tag="xslab")
                        nc.sync.dma_start(
                            out=sl[:, 0:1024],
                            in_=xT[c * 128 : (c + 1) * 128, 0:1024],
                        )
                        slabs.append(sl)
                    for c in range(8):
                        nc.sync.dma_start(
                            out=slabs[c][:, 1024:2048],
                            in_=xT[c * 128 : (c + 1) * 128, 1024:2048],
                        )
                else:
                    slabs = state[b].pop("slabs")
                for tch in range(4):
                    for m, dst in enumerate((qT, kT, vT)):
                        ps = ps1.tile([128, 512], F32, tag="ps1")
                        if b == 0:
                            # cold start is DMA-paced: pad the PE stream with
                            # free dummy matmuls (into this group's own bank,
                            # reset by the c==0 start below) so the HAM clock
                            # gate never sees an idle window and stays at 8/8
                            for _ in range(8 if (tch, m) == (0, 0) else 3):
                                nc.tensor.matmul(
                                    ps,
                                    lhsT=scratch[:, 0:128],
                                    rhs=scratch,
                                    start=True,
                                    stop=True,
                                )
                        for c in range(8):
                            nc.tensor.matmul(
                                ps,
                                lhsT=wq_sb[:, c, m * 128 : (m + 1) * 128],
                                rhs=slabs[c][:, tch * 512 : (tch + 1) * 512],
                                start=(c == 0),
                                stop=(c == 7),
                            )
                        nc.vector.tensor_scalar_add(
                            dst[:, tch * 512 : (tch + 1) * 512], ps, bq_sb[:, m : m + 1]
                        )
                # vT -> V (token-major) + ones column
                va0 = vaug_p.tile([128, 16, 65], DT, tag="va0")
                va1 = vaug_p.tile([128, 16, 65], DT, tag="va1")
                nc.vector.tensor_copy(va0[:, :, 64:65], ones_col)
                nc.vector.tensor_copy(va1[:, :, 64:65], ones_col)
                for tt in range(16):
                    tp = ps1.tile([128, 128], DT, tag="ps1")
                    nc.tensor.transpose(tp, vT[:, tt * 128 : (tt + 1) * 128], id_sb)
                    nc.vector.tensor_copy(va0[:, tt, 0:64], tp[:, 0:64])
                    nc.vector.tensor_copy(va1[:, tt, 0:64], tp[:, 64:128])
                yT = yt_pool.tile([128, T], DT, tag="yT")
                state[b] = {"qT": qT, "kT": kT, "va0": va0, "va1": va1, "yT": yT}

            def emit_kloop(b, qc, filler=None):
                # software-pipelined: AV lags QK by one block so the PE never
                # waits on that block's exp; one filler unit (previous batch's
                # finish work) per block keeps the PE dense while Act paces.
                st = state[b]
                qT, kT, va0, va1 = st["qT"], st["kT"], st["va0"], st["va1"]
                y01 = ps_y.tile([65, 2, 512], F32, tag="y01")
                nkb = 4 * qc + 4
                pts = {}

                def emit_av(kb):
                    pt, lo = pts.pop(kb)
                    nc.tensor.matmul(
                        y01[:, 0, lo:512],
                        lhsT=va0[:, kb, :],
                        rhs=pt[:, 0, lo:512],
                        start=(kb == 0),
                        stop=(kb == nkb - 1),
                    )
                    nc.tensor.matmul(
                        y01[:, 1, lo:512],
                        lhsT=va1[:, kb, :],
                        rhs=pt[:, 1, lo:512],
                        start=(kb == 0),
                        stop=(kb == nkb - 1),
                    )

                for kb in range(nkb):
                    s = ps_s.tile([128, 2, 512], F32, tag="s")
                    # for diagonal blocks only columns q >= (kb-4qc)*128 are
                    # causally reachable; skip the rest entirely (S included)
                    j = max(kb - 4 * qc, 0) if kb >= 4 * qc else 0
                    lo = j * 128
                    nc.tensor.matmul(
                        s[:, 0, lo:512],
                        lhsT=kT[0:64, kb * 128 : (kb + 1) * 128],
                        rhs=qT[0:64, qc * 512 + lo : (qc + 1) * 512],
                        start=True,
                        stop=True,
                    )
                    nc.tensor.matmul(
                        s[:, 1, lo:512],
                        lhsT=kT[64:128, kb * 128 : (kb + 1) * 128],
                        rhs=qT[64:128, qc * 512 + lo : (qc + 1) * 512],
                        start=True,
                        stop=True,
                    )
                    pt = pt_pool.tile([128, 2, 512], DT, tag="pt")
                    nc.scalar.activation(pt[:, :, lo:512], s[:, :, lo:512], EXP, scale=0.125)
                    if kb >= 4 * qc:
                        nc.gpsimd.affine_select(
                            out=pt[:, :, lo : lo + 128],
                            in_=pt[:, :, lo : lo + 128],
                            pattern=[[0, 2], [1, 128]],
                            base=0,
                            channel_multiplier=-1,
                            compare_op=mybir.AluOpType.is_ge,
                            fill=0.0,
                        )
                    pts[kb] = (pt, lo)
                    if filler is not None and kb >= 2:
                        next(filler, None)
                    if kb >= 1:
                        emit_av(kb - 1)
                if filler is not None:
                    next(filler, None)
                emit_av(nkb - 1)
                st[("y", qc)] = y01

            def emit_drain(b, qc):
                # release y PSUM promptly (single DVE copy); ship the
                # denominator row (partition 64) to DRAM — per-batch half
                # bounces re-partition the rows so one reciprocal covers them.
                st = state[b]
                y01 = st.pop(("y", qc))
                yub = yub_p.tile([65, 2, 512], F32, tag="yub")
                nc.vector.tensor_copy(yub, y01)
                nc.sync.dma_start(out=st["dden"][qc], in_=yub[64:65, :, :])
                st[("yub", qc)] = yub

            def emit_gather(b, rows):
                # bounce back from DRAM partition-split, fast-approx
                # reciprocal (~18 correct bits, plenty for the 2e-2 gate)
                st = state[b]
                lo, hi = rows
                n = hi - lo
                dsum = sm_pool.tile([n, 512], F32, tag="dsum")
                nc.sync.dma_start(
                    out=dsum, in_=st["dden"].rearrange("a h q -> (a h) q")[lo:hi]
                )
                rinv = sm_pool.tile([n, 512], F32, tag="rinv")
                nc.vector.reciprocal(rinv, dsum)
                rdt = sm_pool.tile([n, 512], DT, tag="rdt", bufs=4)
                nc.vector.tensor_copy(rdt, rinv)
                st[("rdt", lo)] = rdt

            def _rdt_rows(b, qc):
                st = state[b]
                if b == B - 1 and qc >= 2:
                    return st[("rdt", 4)], 2 * (qc - 2)
                return st[("rdt", 0)], 2 * qc

            def part2_chunks(b, qc):
                # generator form for interleaving into a k-loop: uses only ps1
                # slots (free during attention) so it never contends with the
                # k-loop's s/y PSUM rotation; one proj MM + cast per chunk
                st = state[b]
                yub = st.pop(("yub", qc))
                yT = st["yT"]
                rdt, r0 = _rdt_rows(b, qc)
                n = rdt.shape[0]
                for h in range(2):
                    rb = ps1.tile([64, 512], F32, tag="ps1")
                    nc.tensor.matmul(
                        rb,
                        lhsT=erows[0:n, r0 + h, :],
                        rhs=rdt,
                        start=True,
                        stop=True,
                    )
                    nc.vector.tensor_mul(
                        yT[h * 64 : (h + 1) * 64, qc * 512 : (qc + 1) * 512],
                        yub[0:64, h, :],
                        rb,
                    )
                yield
                osb = None
                for mt in range(8):
                    if mt % 4 == 0:
                        osb = ost_pool.tile(
                            [128, 4, 512], mybir.dt.bfloat16, tag="osb"
                        )
                    o = ps1.tile([128, 512], F32, tag="ps1")
                    nc.tensor.matmul(
                        o,
                        lhsT=wp_sb[:, mt * 128 : (mt + 1) * 128],
                        rhs=yT[:, qc * 512 : (qc + 1) * 512],
                        start=True,
                        stop=True,
                    )
                    # DVE only: an Act cast here would head-of-line block the
                    # enclosing k-loop's exp stream (in-order engine)
                    nc.vector.tensor_copy(osb[:, mt % 4, :], o)
                    if mt % 4 == 3:
                        g = mt // 4
                        nc.sync.dma_start(
                            out=outT[
                                g * 512 : (g + 1) * 512,
                                b * T + qc * 512 : b * T + (qc + 1) * 512,
                            ].rearrange("(a p) q -> p a q", p=128),
                            in_=osb,
                        )
                    yield

            def batch_filler(b):
                # previous batch's four finishes, consumed one unit per
                # k-loop block of the NEXT batch (36 units vs 36 slots)
                for qc in range(4):
                    yield from part2_chunks(b, qc)

            def dummy_filler():
                # batch 0 has no previous finish work; scratch matmuls keep
                # the PE stream dense enough to hold the HAM clock at 8/8
                while True:
                    ps = ps1.tile([128, 512], F32, tag="ps1")
                    nc.tensor.matmul(
                        ps, lhsT=scratch[:, 0:128], rhs=scratch, start=True, stop=True
                    )
                    yield

            def emit_part2(b, qc, tail=False, only=None):
                # tail-only finish: broadcast 1/den across 64 partitions via a
                # K=n matmul, normalize into yT and run the projection chunk.
                # Proj accumulators rotate s/ps1/y01 PSUM slots so the
                # PSUM->bf16 casts (split DVE/Act) never gate the PE.
                st = state[b]
                yT = st["yT"]
                if only != "b":
                    yub = st.pop(("yub", qc))
                    rdt, r0 = _rdt_rows(b, qc)
                    n = rdt.shape[0]
                    rb = ps_s.tile([64, 2, 512], F32, tag="s")
                    for h in range(2):
                        nc.tensor.matmul(
                            rb[:, h, :],
                            lhsT=erows[0:n, r0 + h, :],
                            rhs=rdt,
                            start=True,
                            stop=True,
                        )
                        nc.vector.tensor_mul(
                            yT[h * 64 : (h + 1) * 64, qc * 512 : (qc + 1) * 512],
                            yub[0:64, h, :],
                            rb[:, h, :],
                        )
                if only == "a":
                    return
                slot = [
                    (ps_s, "s"),
                    (ps1, "ps1"),
                    (ps1, "ps1"),
                    (ps_s, "s"),
                    (ps_y, "y01"),
                    (ps1, "ps1"),
                    (ps1, "ps1"),
                    (ps_s, "s"),
                ]
                # the very last finish ships half-groups so the final DMA (and
                # the end-of-kernel drain behind it) starts as early as possible
                per_dma = 2 if (tail and qc == 3) else 4
                for g in range(2):
                    osb = ost_pool.tile([128, 4, 512], mybir.dt.bfloat16, tag="osb")
                    for i in range(4):
                        mt = 4 * g + i
                        pool, tag = slot[mt]
                        o = pool.tile([128, 512], F32, tag=tag)
                        nc.tensor.matmul(
                            o,
                            lhsT=wp_sb[:, mt * 128 : (mt + 1) * 128],
                            rhs=yT[:, qc * 512 : (qc + 1) * 512],
                            start=True,
                            stop=True,
                        )
                        # tail: no exps follow — weight the cast split toward
                        # Act (3/5) since DVE also carries the muls
                        if tail and mt in (1, 3, 5, 6, 7):
                            nc.scalar.copy(osb[:, i, :], o)
                        else:
                            nc.vector.tensor_copy(osb[:, i, :], o)
                        if (i + 1) % per_dma == 0:
                            j = i + 1 - per_dma
                            nc.sync.dma_start(
                                out=outT[
                                    g * 512 + j * 128 : g * 512 + (i + 1) * 128,
                                    b * T + qc * 512 : b * T + (qc + 1) * 512,
                                ].rearrange("(a p) q -> p a q", p=128),
                                in_=osb[:, j : i + 1, :],
                            )

            for b in range(B):
                emit_qkv(b)
                dden = dscr.tile([4, 2, 512], F32, tag="dden")
                state[b]["dden"] = dden
                if b + 1 < B:
                    # pre-emit next batch's x DMAs so they sit ahead of the
                    # proj-out DMAs in the Sync stream (in-order issue engine)
                    nxt = []
                    for c in range(8):
                        sl = xpool.tile([128, 2048], DT, tag="xslab")
                        nc.sync.dma_start(
                            out=sl,
                            in_=xT[c * 128 : (c + 1) * 128, (b + 1) * T : (b + 2) * T],
                        )
                        nxt.append(sl)
                    state[b + 1] = {"slabs": nxt}
                filler = batch_filler(b - 1) if b > 0 else None
                for qc in range(4):
                    emit_kloop(b, qc, filler)
                    emit_drain(b, qc)
                    if b == B - 1 and qc == 1:
                        emit_gather(b, (0, 4))
                if b < B - 1:
                    emit_gather(b, (0, 8))
                if b > 0:
                    # flush any leftover finish units (normally none)
                    for _ in filler:
                        pass
            # tail: interleave the last batch's four finishes (a = bcast+mul,
            # b = proj) so the PE never idles long enough to drop the HAM clock
            emit_part2(B - 1, 0, tail=True, only="a")
            emit_part2(B - 1, 1, tail=True, only="a")
            emit_part2(B - 1, 0, tail=True, only="b")
            emit_gather(B - 1, (4, 8))
            emit_part2(B - 1, 1, tail=True, only="b")
            emit_part2(B - 1, 2, tail=True, only="a")
            emit_part2(B - 1, 2, tail=True, only="b")
            emit_part2(B - 1, 3, tail=True, only="a")
            emit_part2(B - 1, 3, tail=True, only="b")

    _split_waits(nc)
    return nc


_nc_cache = None


def kernel(x, W_qkv, b_qkv, W_proj, b_proj):
    global _nc_cache
    x = np.ascontiguousarray(np.asarray(x, dtype=np.float32))
    W_qkv = np.asarray(W_qkv, dtype=np.float32)
    b_qkv = np.asarray(b_qkv, dtype=np.float32)
    W_proj = np.asarray(W_proj, dtype=np.float32)
    b_proj = np.asarray(b_proj, dtype=np.float32)

    npdt = _op_npdtype()
    xT = np.ascontiguousarray(x.reshape(B * T, C).T).astype(npdt)
    ident = np.eye(128, dtype=np.float32).astype(npdt)
    ehot = np.ascontiguousarray(
        np.broadcast_to(np.eye(8, dtype=np.float32)[:, :, None], (8, 8, 64))
    ).astype(npdt)

    in_maps = []
    for i in range(NCORES):
        s = slice(128 * i, 128 * (i + 1))
        wq = np.ascontiguousarray(
            np.concatenate(
                [W_qkv[:, s], W_qkv[:, 1024:2048][:, s], W_qkv[:, 2048:3072][:, s]],
                axis=1,
            )
        ).astype(npdt)
        bq = np.ascontiguousarray(
            np.stack([b_qkv[0:1024][s], b_qkv[1024:2048][s], b_qkv[2048:3072][s]], axis=1)
        )
        wp = np.ascontiguousarray(W_proj[s, :]).astype(npdt)
        in_maps.append(
            {"xT": xT, "wqkv": wq, "bqkv": bq, "wproj": wp, "ident": ident, "ehot": ehot}
        )

    if _nc_cache is None:
        _nc_cache = build_nc()
    res = run_bass_kernel_spmd(_nc_cache, in_maps, list(range(NCORES)), trace=TRACE)
    kernel.last_result = res

    acc = np.zeros((C, B * T), dtype=np.float32)
    for r in res.results:
        acc += np.asarray(r["outT"], dtype=np.float32)
    out = acc.T.reshape(B, T, C) + b_proj
    return out.astype(np.float32)
